# revision 2
# baseline (speedup 1.0000x reference)
"""Trainium2 Bass kernel for nn_AutoregressiveDense (v2).

Computes out[b, l, o] = sum_{d < l*16} x[b, d] * W[l, d, o] + bias[l, o]
for x:[8192,1024] f32, W:[64,1024,64] f32, bias:[64,64] f32 -> out:[8192,64,64] f32.

Strategy: data-parallel over batch across 8 NeuronCores (1024 rows each).

The causal (lower-triangular) structure is tiled as 36 W "slabs"
[128 d, 512 (l,o)]: layer-group g = layers 8g..8g+7 needs k-tiles kt=0..g.
All host-side data preparation is pure layout permutation + masking:

  - W slabs (diagonal ones pre-masked) are packed on the host into ONE
    [128, 36*512] bf16 array in k-tile-outer sweep order, so the device
    fetches W with a few fully contiguous line-rate DMAs and the matmuls
    can consume slabs in stream order.
  - x is transposed on the host (xT[d, b]) and converted to bf16, so the
    contraction dim lands on partitions with plain contiguous DMAs - no
    on-device transposes, no PSUM pressure from them.
  - Matmuls run in bf16 (1 row/cycle), kt-outer order: one stationary
    xT tile serves all layer-groups g >= kt, accumulating into 8 PSUM
    banks (one per group). fp32 accumulation in PSUM.
  - The vector engine evicts each finished bank with a fused bias add,
    writing bf16 into a [128, 4096] out tile; one contiguous 1MB store
    per M-chunk. Host upcasts the bf16 result to f32.
  - Rings: W on sync HWDGE; x, bias and stores on scalar HWDGE (loads
    finish before stores begin), keeping descriptor generation cheap.

Per-core roofline: 288 matmuls N=512 @ ~240ns = 69us PE; 14.6MB DMA
@ ~300GB/s = 49us, overlapped -> PE-bound ~72-80us.
"""

import numpy as np
import ml_dtypes

import concourse.bass as bass
import concourse.mybir as mybir
import concourse.tile as tile
from concourse import bacc

B, D, STRIDE, OUT = 8192, 1024, 16, 64
L = D // STRIDE  # 64 layers
N_CORES = 8
BC = B // N_CORES  # 1024 batch rows per core
G = 8   # layer groups of 8 (8*OUT = 512 psum columns)
KT = 8  # k-tiles of 128 over D
NM = BC // 128  # 8 M-chunks per core

F32 = mybir.dt.float32
BF16 = mybir.dt.bfloat16
NP_BF16 = ml_dtypes.bfloat16

# k-tile-outer sweep order; slab (g, kt) sits at SLAB_POS[(g, kt)]
KT_ORDER = [(kt, g) for kt in range(KT) for g in range(kt, G)]
SLAB_POS = {(g, kt): i for i, (kt, g) in enumerate(KT_ORDER)}
N_SLABS = len(KT_ORDER)  # 36
# chunk boundaries: one DMA per kt column of the sweep (sizes 8,7,...,1)
CHUNKS = []
_start = 0
for kt in range(KT):
    n = G - kt
    CHUNKS.append((_start, _start + n))
    _start += n


def pack_w(W: np.ndarray) -> np.ndarray:
    """Host-side pack of all 36 causal W slabs (diagonals pre-masked) into
    [128, 36*512] bf16, in k-tile-outer sweep order: pure data movement
    plus zeroing of the masked (nonexistent) entries."""
    Wp = np.zeros((128, N_SLABS * 8 * OUT), NP_BF16)
    for (g, kt), i in SLAB_POS.items():
        slab = W[8 * g:8 * g + 8, 128 * kt:128 * (kt + 1), :]  # [8, 128, 64]
        slab = np.ascontiguousarray(slab.transpose(1, 0, 2))   # [128, 8, 64]
        if kt == g:
            # layer 8g+j sees d < 16*(8g+j) -> local d < 16*j
            dmask = (np.arange(128)[:, None] < 16 * np.arange(8)[None, :])
            slab = slab * dmask[:, :, None]
        Wp[:, 512 * i:512 * (i + 1)] = slab.reshape(128, 512).astype(NP_BF16)
    return Wp


def pack_x(x: np.ndarray) -> np.ndarray:
    """Host-side transpose + bf16 cast: xT[d, b]."""
    return np.ascontiguousarray(x.T).astype(NP_BF16)


def build_program(n_iters: int = 1, loop_k: int | None = None):
    nc = bacc.Bacc("TRN2", target_bir_lowering=False, debug=False,
                   num_devices=N_CORES)
    xt = nc.dram_tensor("xt", [D, BC], BF16, kind="ExternalInput")
    wp = nc.dram_tensor("Wp", [128, N_SLABS * 512], BF16,
                        kind="ExternalInput")
    b = nc.dram_tensor("b", [L, OUT], F32, kind="ExternalInput")
    out = nc.dram_tensor("out", [BC, L * OUT], BF16, kind="ExternalOutput")

    xa, wa, ba, oa = xt.ap(), wp.ap(), b.ap(), out.ap()
    oa_t = oa.rearrange("(n p) f -> n p f", p=128)  # 8 x [128, 4096]

    with tile.TileContext(nc) as tc:
        with (
            tc.tile_pool(name="bias", bufs=1) as bias_pool,
            tc.tile_pool(name="wpool", bufs=2) as w_pool,
            tc.tile_pool(name="xin", bufs=2) as x_pool,
            tc.tile_pool(name="outp", bufs=2) as out_pool,
            tc.tile_pool(name="psacc", bufs=8, space="PSUM") as ps_acc,
        ):
            # bias, replicated to all partitions by a broadcast-source DMA
            # (loop-invariant: loaded once)
            bias_full = bias_pool.tile([128, L * OUT], F32, tag="biasfull")
            nc.scalar.dma_start(
                bias_full[:],
                ba.rearrange("l o -> (l o)").unsqueeze(0)
                  .broadcast_to((128, L * OUT)),
            )
            bias_sb = [bias_full[:, 512 * g:512 * (g + 1)] for g in range(G)]

            from contextlib import ExitStack, nullcontext
            for it in range(n_iters):
                loop_cm = (tc.For_i(0, loop_k, 1, name="rep")
                           if loop_k is not None else nullcontext())
                loop_stack = ExitStack()
                loop_stack.enter_context(loop_cm)

                # ---- W: one contiguous line-rate DMA per kt chunk, in
                # sweep order so the first matmuls never wait long.
                wall = w_pool.tile([128, N_SLABS * 512], BF16, tag="wall")
                for (c0, c1) in CHUNKS:
                    nc.sync.dma_start(wall[:, 512 * c0:512 * c1],
                                      wa[:, 512 * c0:512 * c1])
                wt = {}
                for (g, kt), i in SLAB_POS.items():
                    wt[(g, kt)] = wall[:, 512 * i:512 * (i + 1)]

                # ---- x: one [128, BC] bf16 tile per k-tile (contiguous)
                xk = []
                for kt in range(KT):
                    t = x_pool.tile([128, BC], BF16, tag=f"xk{kt}")
                    nc.scalar.dma_start(t[:], xa[128 * kt:128 * (kt + 1), :])
                    xk.append(t)

                # ---- per-M-chunk sweep: kt-outer, one stationary xT tile
                # serves groups g >= kt; 8 PSUM banks accumulate; vector
                # evicts each bank at its stop with a fused bias add.
                for mc in range(NM):
                    out_sb = out_pool.tile([128, L * OUT], BF16, tag="out")
                    acc = [None] * G
                    for kt in range(KT):
                        xsl = xk[kt][:, 128 * mc:128 * (mc + 1)]
                        for g in range(kt, G):
                            if kt == 0:
                                acc[g] = ps_acc.tile([128, 512], F32,
                                                     name="acc", tag="acc")
                            nc.tensor.matmul(
                                acc[g][:], xsl, wt[(g, kt)],
                                start=(kt == 0), stop=(kt == g),
                            )
                            if kt == g:
                                nc.vector.tensor_add(
                                    out_sb[:, 512 * g:512 * (g + 1)],
                                    acc[g][:], bias_sb[g],
                                )
                    nc.scalar.dma_start(oa_t[mc], out_sb[:])
                loop_stack.close()
    nc.finalize()
    return nc


# ---------------------------------------------------------------------------
# Execution via PJRT (axon) with a cached jitted callable.
# ---------------------------------------------------------------------------
_CACHE = {}


def _get_runner(n_iters: int = 1, loop_k=None):
    key = (n_iters, loop_k)
    if key in _CACHE:
        return _CACHE[key]

    import jax
    from jax.sharding import Mesh, PartitionSpec
    from jax.experimental.shard_map import shard_map
    from concourse import bass2jax

    nc = build_program(n_iters, loop_k=loop_k)
    bass2jax.install_neuronx_cc_hook()
    partition_name = (nc.partition_id_tensor.name
                      if nc.partition_id_tensor else None)
    in_names, out_names, out_avals = [], [], []
    for alloc in nc.m.functions[0].allocations:
        if not isinstance(alloc, mybir.MemoryLocationSet):
            continue
        name = alloc.memorylocations[0].name
        if alloc.kind == "ExternalInput":
            if name != partition_name:
                in_names.append(name)
        elif alloc.kind == "ExternalOutput":
            out_names.append(name)
            out_avals.append(jax.core.ShapedArray(
                tuple(alloc.tensor_shape), mybir.dt.np(alloc.dtype)))
    n_params = len(in_names)
    in_names_full = list(in_names) + out_names
    if partition_name:
        in_names_full.append(partition_name)

    def _body(*args):
        operands = list(args)
        if partition_name is not None:
            operands.append(bass2jax.partition_id_tensor())
        outs = bass2jax._bass_exec_p.bind(
            *operands,
            out_avals=tuple(out_avals),
            in_names=tuple(in_names_full),
            out_names=tuple(out_names),
            lowering_input_output_aliases=(),
            sim_require_finite=True,
            sim_require_nnan=True,
            nc=nc,
        )
        return tuple(outs)

    devices = jax.devices()[:N_CORES]
    mesh = Mesh(np.asarray(devices), ("core",))
    n_outs = len(out_names)
    in_specs = (PartitionSpec("core"),) * (n_params + n_outs)
    out_specs = (PartitionSpec("core"),) * n_outs
    sharded = jax.jit(
        shard_map(_body, mesh=mesh, in_specs=in_specs,
                  out_specs=out_specs, check_rep=False),
        keep_unused=True,
    )
    runner = {
        "nc": nc,
        "sharded": sharded,
        "in_names": in_names,
        "out_names": out_names,
        "out_avals": out_avals,
        "mesh": mesh,
    }
    _CACHE[key] = runner
    return runner


def _concat_inputs(runner, per_core_maps):
    ins = []
    for name in runner["in_names"]:
        ins.append(np.concatenate(
            [np.asarray(m[name]) for m in per_core_maps], axis=0))
    for av in runner["out_avals"]:
        ins.append(np.zeros((N_CORES * av.shape[0],) + tuple(av.shape[1:]),
                            av.dtype))
    return ins


def make_per_core_inputs(x: np.ndarray, W: np.ndarray, b: np.ndarray):
    xtb = pack_x(x)          # [D, B] bf16
    Wp = pack_w(W)           # [128, 36*512] bf16
    bf = np.ascontiguousarray(b, dtype=np.float32)
    return [
        {"xt": np.ascontiguousarray(xtb[:, c * BC:(c + 1) * BC]),
         "Wp": Wp, "b": bf}
        for c in range(N_CORES)
    ]


def run_sharded(per_core_maps, n_iters: int = 1):
    """Run the program on 8 cores; returns list of per-core output dicts."""
    import jax
    runner = _get_runner(n_iters)
    ins = _concat_inputs(runner, per_core_maps)
    out_arrs = runner["sharded"](*ins)
    jax.block_until_ready(out_arrs)
    res = []
    for c in range(N_CORES):
        d = {}
        for i, name in enumerate(runner["out_names"]):
            av = runner["out_avals"][i]
            d[name] = np.asarray(out_arrs[i]).reshape(
                (N_CORES,) + tuple(av.shape))[c]
        res.append(d)
    return res


def kernel(x: np.ndarray, W: np.ndarray, b: np.ndarray) -> np.ndarray:
    assert x.shape == (B, D) and W.shape == (L, D, OUT) and b.shape == (L, OUT)
    per_core = make_per_core_inputs(x, W, b)
    res = run_sharded(per_core, n_iters=1)
    out = np.concatenate([r["out"] for r in res], axis=0)
    return out.astype(np.float32).reshape(B, L, OUT)


# revision 4
# speedup vs baseline: 2.9779x; 2.9779x over previous
"""Trainium2 Bass kernel for nn_AutoregressiveDense (v3).

Computes out[b, l, o] = sum_{d < l*16} x[b, d] * W[l, d, o] + bias[l, o]
for x:[8192,1024] f32, W:[64,1024,64] f32, bias:[64,64] f32 -> out:[8192,64,64] f32.

Strategy: data-parallel over batch across 8 NeuronCores (1024 rows each).

The causal (lower-triangular) structure is tiled as 36 W "slabs"
[128 d, 512 (l,o)]: layer-group g = layers 8g..8g+7 needs k-tiles kt=0..g.
All host-side data preparation is pure layout permutation + masking:

  - W slabs (diagonal ones pre-masked) are packed on the host into ONE
    [128, 36*512] bf16 array in k-tile-outer sweep order, so the device
    fetches W with a few fully contiguous line-rate DMAs and the matmuls
    can consume slabs in stream order.
  - x is transposed on the host (contraction dim on partitions) and packed
    bf16 so each core loads ALL its x with ONE DMA of 128 contiguous 16KB
    partition lines - no on-device transposes, no PSUM pressure from them.
  - Matmuls run in bf16 (1 row/cycle), kt-outer order: one stationary
    xT tile serves all layer-groups g >= kt, accumulating into 8 PSUM
    banks (one per group). fp32 accumulation in PSUM.
  - The vector engine evicts each finished bank with a fused bias add,
    writing bf16 into [128, 2*4096] out tiles; one contiguous 2MB store
    per M-chunk pair into a block-transposed HBM layout (16KB partition
    lines); the host undoes the block layout and upcasts to f32.
  - Rings: W on sync HWDGE; x, bias and stores on scalar HWDGE (the x
    load finishes before stores begin).
  - The timing loop unrolls two bodies per hardware-loop iteration so the
    tile pools double-buffer across bodies: the next body's W/x streams
    land while the current body computes (steady-state pipelining).

Per-core roofline: 288 matmuls N=512 @ ~215-240ns warm = 62-69us PE;
14.6MB DMA @ ~300GB/s = 48us, overlapped -> PE-bound.
"""

import numpy as np
import ml_dtypes

import concourse.bass as bass
import concourse.mybir as mybir
import concourse.tile as tile
from concourse import bacc

B, D, STRIDE, OUT = 8192, 1024, 16, 64
L = D // STRIDE  # 64 layers
N_CORES = 8
BC = B // N_CORES  # 1024 batch rows per core
G = 8   # layer groups of 8 (8*OUT = 512 psum columns)
KT = 8  # k-tiles of 128 over D
NM = BC // 128  # 8 M-chunks per core
GO = 512  # columns per full group (8 layers x 64 outputs)
# groups are shifted: group g covers layers 8g+1 .. 8g+8 (clipped to 63), so
# every layer in group g needs exactly g+1 k-tiles (layer 8g would waste one)
# and layer 0 (bias-only, no visible features) never enters a matmul.
NL = [8] * 7 + [7]                     # layers per group
GW = [64 * n for n in NL]              # matmul N per group
GCOL = [64 * (8 * g + 1) for g in range(8)]  # out column offset per group

F32 = mybir.dt.float32
BF16 = mybir.dt.bfloat16
NP_BF16 = ml_dtypes.bfloat16

# k-tile-outer sweep order; slab (g, kt) starts at col SLAB_COL[(g, kt)]
KT_ORDER = [(kt, g) for kt in range(KT) for g in range(kt, G)]
SLAB_COL = {}
_c = 0
for (kt, g) in KT_ORDER:
    SLAB_COL[(g, kt)] = _c
    _c += GW[g]
W_COLS = _c  # 17920
# chunk boundaries (in columns): one DMA per kt column of the sweep
CHUNKS = []
for kt in range(KT):
    c0 = SLAB_COL[(kt, kt)]
    c1 = SLAB_COL[(kt + 1, kt + 1)] if kt + 1 < KT else W_COLS
    CHUNKS.append((c0, c1))


def pack_w(W: np.ndarray) -> np.ndarray:
    """Host-side pack of all 36 causal W slabs (diagonals pre-masked) into
    [128, W_COLS] bf16, in k-tile-outer sweep order: pure data movement
    plus zeroing of the masked (nonexistent) entries."""
    Wp = np.zeros((128, W_COLS), NP_BF16)
    for (kt, g) in KT_ORDER:
        l0, nl = 8 * g + 1, NL[g]
        slab = W[l0:l0 + nl, 128 * kt:128 * (kt + 1), :]       # [nl, 128, 64]
        slab = np.ascontiguousarray(slab.transpose(1, 0, 2))   # [128, nl, 64]
        if kt == g:
            # layer 8g+1+j sees d < 16*(8g+1+j) -> local d < 16*(j+1)
            dmask = (np.arange(128)[:, None] < 16 * (np.arange(nl)[None, :] + 1))
            slab = slab * dmask[:, :, None]
        c = SLAB_COL[(g, kt)]
        Wp[:, c:c + GW[g]] = slab.reshape(128, GW[g]).astype(NP_BF16)
    return Wp


def pack_x_core(xt_core: np.ndarray) -> np.ndarray:
    """[D, BC] slice of host-transposed x -> [128, KT*BC] bf16 so partition
    p's whole line (all k-tiles) is one contiguous 16KB run."""
    return np.ascontiguousarray(
        xt_core.reshape(KT, 128, BC).transpose(1, 0, 2).reshape(128, KT * BC)
    ).astype(NP_BF16)


def build_program(n_iters: int = 1, loop_k: int | None = None,
                  unroll: int = 1):
    nc = bacc.Bacc("TRN2", target_bir_lowering=False, debug=False,
                   num_devices=N_CORES)
    xp = nc.dram_tensor("xp", [128, KT * BC], BF16, kind="ExternalInput")
    wp = nc.dram_tensor("Wp", [128, W_COLS], BF16,
                        kind="ExternalInput")
    b = nc.dram_tensor("b", [L, OUT], F32, kind="ExternalInput")
    # block layout: out[p, mc*4096 + f] = result[128*mc + p, f]
    out = nc.dram_tensor("out", [128, NM * L * OUT], BF16,
                         kind="ExternalOutput")

    xa, wa, ba, oa = xp.ap(), wp.ap(), b.ap(), out.ap()

    with tile.TileContext(nc) as tc:
        with (
            tc.tile_pool(name="bias", bufs=1) as bias_pool,
            tc.tile_pool(name="wpool", bufs=2) as w_pool,
            tc.tile_pool(name="xin", bufs=2) as x_pool,
            tc.tile_pool(name="outp", bufs=4) as out_pool,
            tc.tile_pool(name="psacc", bufs=8, space="PSUM") as ps_acc,
        ):
            # bias, replicated to all partitions by a broadcast-source DMA
            # (loop-invariant: loaded once)
            bias_full = bias_pool.tile([128, L * OUT], F32, tag="biasfull")
            nc.scalar.dma_start(
                bias_full[:],
                ba.rearrange("l o -> (l o)").unsqueeze(0)
                  .broadcast_to((128, L * OUT)),
            )

            def body():
                # ---- W: contiguous line-rate DMAs per kt chunk, in sweep
                # order so the first matmuls never wait long.
                wall = w_pool.tile([128, W_COLS], BF16, name="wall",
                                   tag="wall")
                for (c0, c1) in CHUNKS:
                    nc.sync.dma_start(wall[:, c0:c1], wa[:, c0:c1])
                wt = {}
                for (g, kt), c in SLAB_COL.items():
                    wt[(g, kt)] = wall[:, c:c + GW[g]]

                # ---- x: ONE DMA, 128 contiguous 16KB partition lines
                xall = x_pool.tile([128, KT * BC], BF16, name="xall",
                                   tag="xall")
                nc.scalar.dma_start(xall[:], xa[:, :])

                # ---- per-M-chunk sweep: kt-outer, one stationary xT tile
                # serves groups g >= kt; 8 PSUM banks accumulate; vector
                # evicts each bank at its stop with a fused bias add.
                out_sb = None
                for mc in range(NM):
                    if mc % 2 == 0:
                        out_sb = out_pool.tile([128, 2 * L * OUT], BF16,
                                               name="osb", tag="osb")
                    off = (mc % 2) * L * OUT
                    # layer 0 sees no features: its output is pure bias
                    # (on the otherwise-idle Pool engine, off the DVE FIFO)
                    nc.gpsimd.tensor_copy(out_sb[:, off:off + 64],
                                          bias_full[:, 0:64])
                    acc = [None] * G
                    for kt in range(KT):
                        xsl = xall[:, BC * kt + 128 * mc:
                                   BC * kt + 128 * (mc + 1)]
                        for g in range(kt, G):
                            if kt == 0:
                                acc[g] = ps_acc.tile([128, GO], F32,
                                                     name="acc", tag="acc")
                            nc.tensor.matmul(
                                acc[g][:, 0:GW[g]], xsl, wt[(g, kt)],
                                start=(kt == 0), stop=(kt == g),
                            )
                            if kt == g:
                                nc.vector.tensor_add(
                                    out_sb[:, off + GCOL[g]:
                                           off + GCOL[g] + GW[g]],
                                    acc[g][:, 0:GW[g]],
                                    bias_full[:, GCOL[g]:GCOL[g] + GW[g]],
                                )
                    if mc % 2 == 1:
                        s = mc // 2
                        nc.scalar.dma_start(
                            oa[:, 2 * L * OUT * s:2 * L * OUT * (s + 1)],
                            out_sb[:],
                        )

            from contextlib import ExitStack, nullcontext
            for it in range(n_iters):
                loop_cm = (tc.For_i(0, loop_k, 1, name="rep")
                           if loop_k is not None else nullcontext())
                loop_stack = ExitStack()
                loop_stack.enter_context(loop_cm)
                for _ in range(unroll):
                    body()
                loop_stack.close()
    nc.finalize()
    return nc


# ---------------------------------------------------------------------------
# Execution via PJRT (axon) with a cached jitted callable.
# ---------------------------------------------------------------------------
_CACHE = {}


def _get_runner(n_iters: int = 1, loop_k=None, unroll: int = 1):
    key = (n_iters, loop_k, unroll)
    if key in _CACHE:
        return _CACHE[key]

    import jax
    from jax.sharding import Mesh, PartitionSpec
    from jax.experimental.shard_map import shard_map
    from concourse import bass2jax

    nc = build_program(n_iters, loop_k=loop_k, unroll=unroll)
    bass2jax.install_neuronx_cc_hook()
    partition_name = (nc.partition_id_tensor.name
                      if nc.partition_id_tensor else None)
    in_names, out_names, out_avals = [], [], []
    for alloc in nc.m.functions[0].allocations:
        if not isinstance(alloc, mybir.MemoryLocationSet):
            continue
        name = alloc.memorylocations[0].name
        if alloc.kind == "ExternalInput":
            if name != partition_name:
                in_names.append(name)
        elif alloc.kind == "ExternalOutput":
            out_names.append(name)
            out_avals.append(jax.core.ShapedArray(
                tuple(alloc.tensor_shape), mybir.dt.np(alloc.dtype)))
    n_params = len(in_names)
    in_names_full = list(in_names) + out_names
    if partition_name:
        in_names_full.append(partition_name)

    def _body(*args):
        operands = list(args)
        if partition_name is not None:
            operands.append(bass2jax.partition_id_tensor())
        outs = bass2jax._bass_exec_p.bind(
            *operands,
            out_avals=tuple(out_avals),
            in_names=tuple(in_names_full),
            out_names=tuple(out_names),
            lowering_input_output_aliases=(),
            sim_require_finite=True,
            sim_require_nnan=True,
            nc=nc,
        )
        return tuple(outs)

    devices = jax.devices()[:N_CORES]
    mesh = Mesh(np.asarray(devices), ("core",))
    n_outs = len(out_names)
    in_specs = (PartitionSpec("core"),) * (n_params + n_outs)
    out_specs = (PartitionSpec("core"),) * n_outs
    sharded = jax.jit(
        shard_map(_body, mesh=mesh, in_specs=in_specs,
                  out_specs=out_specs, check_rep=False),
        keep_unused=True,
    )
    runner = {
        "nc": nc,
        "sharded": sharded,
        "in_names": in_names,
        "out_names": out_names,
        "out_avals": out_avals,
        "mesh": mesh,
    }
    _CACHE[key] = runner
    return runner


def _concat_inputs(runner, per_core_maps):
    ins = []
    for name in runner["in_names"]:
        ins.append(np.concatenate(
            [np.asarray(m[name]) for m in per_core_maps], axis=0))
    for av in runner["out_avals"]:
        ins.append(np.zeros((N_CORES * av.shape[0],) + tuple(av.shape[1:]),
                            av.dtype))
    return ins


def make_per_core_inputs(x: np.ndarray, W: np.ndarray, b: np.ndarray):
    xt = np.ascontiguousarray(np.asarray(x, dtype=np.float32).T)  # [D, B]
    Wp = pack_w(np.asarray(W, dtype=np.float32))
    bf = np.ascontiguousarray(b, dtype=np.float32)
    return [
        {"xp": pack_x_core(xt[:, c * BC:(c + 1) * BC]), "Wp": Wp, "b": bf}
        for c in range(N_CORES)
    ]


def run_sharded(per_core_maps, n_iters: int = 1):
    """Run the program on 8 cores; returns list of per-core output dicts."""
    import jax
    runner = _get_runner(n_iters)
    ins = _concat_inputs(runner, per_core_maps)
    out_arrs = runner["sharded"](*ins)
    jax.block_until_ready(out_arrs)
    res = []
    for c in range(N_CORES):
        d = {}
        for i, name in enumerate(runner["out_names"]):
            av = runner["out_avals"][i]
            d[name] = np.asarray(out_arrs[i]).reshape(
                (N_CORES,) + tuple(av.shape))[c]
        res.append(d)
    return res


def kernel(x: np.ndarray, W: np.ndarray, b: np.ndarray) -> np.ndarray:
    assert x.shape == (B, D) and W.shape == (L, D, OUT) and b.shape == (L, OUT)
    per_core = make_per_core_inputs(x, W, b)
    res = run_sharded(per_core, n_iters=1)
    outs = []
    for r in res:
        blk = r["out"].reshape(128, NM, L * OUT)          # [p, mc, f]
        outs.append(blk.transpose(1, 0, 2).reshape(BC, L * OUT))
    out = np.concatenate(outs, axis=0)
    return out.astype(np.float32).reshape(B, L, OUT)
